# revision 88
# baseline (speedup 1.0000x reference)
"""Two-layer GCN (GCNConv x2 + log_softmax) on 8 Trainium2 NeuronCores.

Strategy (graph/data parallel, nodes sharded 8 ways; v2 "gather-x" design):
  - No hidden gather table for layer 1: per-edge messages are the pre-scaled
    input rows xs = dinv*x themselves ([100k, 128] bf16 = 256B rows).  The
    aggregation matmul accumulates acc[in_feat, dst] += X_col^T @ S_col and
    W1 is applied once per 512-dst group AFTER aggregation
    (A @ X) @ W1 == A @ (X @ W1).
  - Self-loops (and the handful of natural src==dst edges) never enter the
    gather stream; their contribution mult[d]*dinv[d]*h[d] is added in the
    epilogues from locally-available rows.
  - Gather tables are viewed as [n/2, 512B] pair-rows (int16 indices then
    need only 2 src windows).  Edge slots are bucketed per (3-fat-group
    chunk, pair-row window), sorted by (pair parity, fat, local dst id),
    densely packed per core, and padded only to the max-over-cores column
    count (static SPMD program; per-core idx/lid data).  Columns may cross
    parity / fat-group / 128-lid-window boundaries; each touched
    (parity, fat, lid-window) gets one S subcolumn reading its half of the
    512B slot (static union over cores).  Transfers split into <=32-column
    sub-calls to bound SBUF; idx loads once per block.
  - S columns are built with tensor_tensor is_equal against an iota row using
    a pair-duplicated lid stream so every operand has innermost stride 1 and
    the DVE 2x perf mode applies.
  - Layer 1 accumulates feature-major ([128 in, 512 dst] f32, one PSUM bank
    per fat group); epilogue: W1^T matmul, +self, *dinv, relu+b1, *dinv ->
    t2p; W2 matmul -> node-major h2 staging -> AllGather of compact
    [12500, 40] fp8e4m3 slices; a local expand pipeline (DMA in, DVE
    fp8->bf16 convert, DMA out) builds the 256B-row bf16 layer-2 table.
  - Layer 2 accumulates node-major ([128 dst, 4x64] f32, half a PSUM bank per
    fat group); per-chunk epilogues do *dinv, +self, +b2 and the log_softmax
    through exp-sum (interleaved with aggregation); a single deferred Ln +
    subtract finishes and stores the output.
"""

import math
import os
from contextlib import ExitStack
from dataclasses import dataclass

import numpy as np
import ml_dtypes

import concourse.bass as bass
import concourse.tile as tile
from concourse import bacc, mybir
from concourse.bass_utils import run_bass_kernel_spmd

F32 = mybir.dt.float32
BF16 = mybir.dt.bfloat16
FP8 = mybir.dt.float8e4
I16 = mybir.dt.int16
AF = mybir.ActivationFunctionType
ALU = mybir.AluOpType


@dataclass
class Cfg:
    n: int = 100000        # nodes
    nin: int = 128         # input features
    hid: int = 64          # hidden features
    outf: int = 40         # output features
    ncores: int = 8
    nwin: int = 2          # src pair-row windows (int16 idx range)
    pairw: int = 32768     # 512B pair-rows per src window
    qcols: int = 32        # max columns per gather sub-call (SBUF cap)
    g: int = 128           # slot column height / lid window
    gf: int = 512          # fat dst group (PSUM bank granularity)
    chunk_f: int = 3       # fat groups per gather-call chunk
    sub: int = 24          # S subcolumns per build slab

    @property
    def per(self):
        return self.n // self.ncores

    @property
    def win(self):
        return self.n // self.nwin

    @property
    def ngf(self):
        return math.ceil(self.per / self.gf)

    @property
    def nlw(self):
        return self.gf // self.g

    @property
    def nreg(self):
        # 128-node regions covering per (rounded up to fat-group multiples)
        return self.ngf * self.nlw

    @property
    def perp(self):
        return self.nreg * self.g


# ---------------------------------------------------------------- host side


def _preprocess(x, edge_index, W1, b1, W2, b2, cfg: Cfg):
    n, per, g, gf, win = cfg.n, cfg.per, cfg.g, cfg.gf, cfg.win
    nc_, ngf, nwin, nlw = cfg.ncores, cfg.ngf, cfg.nwin, cfg.nlw

    src0 = edge_index[0].astype(np.int64)
    dst0 = edge_index[1].astype(np.int64)

    # degree includes the reference's appended self-loops
    deg = (np.bincount(dst0, minlength=n) + 1).astype(np.float64)
    dinv = (1.0 / np.sqrt(deg)).astype(np.float32)
    # natural self-loops fold into the analytic self term
    mult = np.ones(n, dtype=np.float32)
    selfm = src0 == dst0
    np.add.at(mult, dst0[selfm], 1.0)

    xs = (x * dinv[:, None]).astype(ml_dtypes.bfloat16)  # gather table rows

    cross = ~selfm
    src = src0[cross]
    dst = dst0[cross]

    core = dst // per
    fat = (dst % per) // gf
    pairw = cfg.pairw           # pair-rows per src window (int16 range)
    wsrc = (src // 2) // pairw  # src window over 512B pair-rows
    par = src % 2               # which half of the pair row
    lid = (dst % per) % gf

    # chunks of fat groups; each gather call covers one (chunk, src window)
    # with its slots densely packed in (fat, lid) order -- columns may cross
    # fat-group boundaries (each touched (fat, lid-window) gets a subcolumn)
    chunks = [
        list(range(k0, min(k0 + cfg.chunk_f, ngf)))
        for k0 in range(0, ngf, cfg.chunk_f)
    ]
    nch = len(chunks)
    k0s = np.zeros(ngf, dtype=np.int64)
    ckid_of_fat = np.zeros(ngf, dtype=np.int64)
    for ki, K in enumerate(chunks):
        for f in K:
            ckid_of_fat[f] = ki
            k0s[f] = K[0]
    ckid = ckid_of_fat[fat]

    # counts per (core, chunk, wsrc) -> shared column counts (max over cores)
    ccounts = np.zeros((nc_, nch, nwin), dtype=np.int64)
    np.add.at(ccounts, (core, ckid, wsrc), 1)
    cols_cw = np.ceil(ccounts.max(axis=0) / g).astype(np.int64)  # [nch, nwin]
    assert (ccounts > 0).all(), "empty (core,chunk,wsrc) bucket"

    # per-core cumulative (parity, fat-in-chunk, lid-window) run boundaries
    nsb1 = cfg.chunk_f * nlw
    nsb = 2 * nsb1
    cnt_sub = np.zeros((nc_, nch, nwin, nsb), dtype=np.int64)
    np.add.at(
        cnt_sub,
        (core, ckid, wsrc, par * nsb1 + (fat - k0s[fat]) * nlw + lid // g),
        1,
    )
    cum_sub = np.zeros((nc_, nch, nwin, nsb + 1), dtype=np.int64)
    cum_sub[..., 1:] = np.cumsum(cnt_sub, axis=-1)

    blk_off = {}
    off = 0
    for ki, K in enumerate(chunks):
        for w in range(nwin):
            blk_off[(ki, w)] = off
            off += int(cols_cw[ki, w]) * g
    tot_slots = off
    tot_cols = tot_slots // g

    # subcolumn structure: per (chunk, wsrc, col): sorted list of (fat, lw)
    # (union over cores of runs overlapping the column's slot range)
    sub_ws = {}
    real_rows = [min(gf, per - f * gf) for f in range(ngf)]
    nsub_tot = 0
    for ki, K in enumerate(chunks):
        for w in range(nwin):
            ncol = int(cols_cw[ki, w])
            for j in range(ncol):
                lo, hi = j * g, j * g + g
                ws = set()
                for c in range(nc_):
                    cc = cum_sub[c, ki, w]
                    for sb_ in range(nsb):
                        if (
                            cc[sb_] < hi
                            and cc[sb_ + 1] > lo
                            and cc[sb_ + 1] > cc[sb_]
                        ):
                            sb1 = sb_ % nsb1
                            ws.add(
                                (sb_ // nsb1, K[0] + sb1 // nlw, sb1 % nlw)
                            )
                wl = sorted(ws)
                assert wl, (ki, w, j)
                sub_ws[(ki, w, j)] = wl
                nsub_tot += len(wl)
    # every real region must receive at least one matmul (PSUM init)
    covered = set()
    for key_, wl in sub_ws.items():
        covered.update((f, lw) for po, f, lw in wl)
    for f in range(ngf):
        for lw in range(math.ceil(real_rows[f] / g)):
            assert (f, lw) in covered, (f, lw)

    # ---- per-core idx / fat / lid arrays (dense packing) ----
    order = np.lexsort((lid, fat, par, wsrc, ckid, core))
    src_s = src[order]
    core_s, fat_s, w_s, lid_s = core[order], fat[order], wsrc[order], lid[order]
    par_s = par[order]
    ck_s = ckid_of_fat[fat_s]

    call_base = blk_off

    idx_all = np.zeros((nc_, tot_slots), dtype=np.int16)
    lid_all = np.full((nc_, tot_slots), 4 * g, dtype=np.int64)  # pad sentinel
    fat_all = np.full((nc_, tot_slots), -1, dtype=np.int64)
    par_all = np.full((nc_, tot_slots), -1, dtype=np.int64)
    for c in range(nc_):
        m = core_s == c
        sc_, fc, wc, lc, kc = src_s[m], fat_s[m], w_s[m], lid_s[m], ck_s[m]
        pc = par_s[m]
        key = kc * nwin + wc
        change = np.r_[True, key[1:] != key[:-1]]
        run_id = np.cumsum(change) - 1
        run_start = np.flatnonzero(change)
        rank = np.arange(len(key)) - run_start[run_id]
        base = np.array(
            [call_base[(kk, ww)] for kk, ww in zip(kc[change], wc[change])]
        )
        slot = base[run_id] + rank
        idx_all[c, slot] = (sc_ // 2 - wc * pairw).astype(np.int16)
        lid_all[c, slot] = lc
        fat_all[c, slot] = fc
        par_all[c, slot] = pc

    # idx wrapped [128, tot_slots//16] (16-partition wrap, replicated x8)
    idx_wrap = np.zeros((nc_, 128, tot_slots // 16), dtype=np.int16)
    for c in range(nc_):
        wrapped = idx_all[c].reshape(-1, 16).T
        idx_wrap[c] = np.tile(wrapped, (8, 1))

    # lid2 stream: per subcol (ordered like the program consumes them):
    # 128 lidloc values pair-duplicated -> [128, 2*nsub_tot]
    lid2 = np.zeros((nc_, 128, 2 * nsub_tot), dtype=ml_dtypes.bfloat16)
    call_meta = []   # per sub-call: (sl0, nsl, w2)
    mm_meta = []     # per sub-call: [(colpos, lw, fat, subidx, parity)]
    chunk_entries = [[] for _ in chunks]
    sidx = 0
    for ki, K in enumerate(chunks):
        for w in range(nwin):
            sl0 = blk_off[(ki, w)]
            ncol = int(cols_cw[ki, w])
            mlist = []
            for j in range(ncol):
                slot0 = sl0 + j * g
                col_lids = lid_all[:, slot0 : slot0 + g]  # [nc, 128]
                col_fats = fat_all[:, slot0 : slot0 + g]
                col_pars = par_all[:, slot0 : slot0 + g]
                for po, f, lw in sub_ws[(ki, w, j)]:
                    ll = np.where(
                        (col_pars == po)
                        & (col_fats == f)
                        & (col_lids >= lw * g)
                        & (col_lids < (lw + 1) * g),
                        col_lids - lw * g,
                        255,
                    )
                    v = ll.astype(ml_dtypes.bfloat16)  # [nc, 128]
                    lid2[:, :, 2 * sidx] = v
                    lid2[:, :, 2 * sidx + 1] = v
                    mlist.append((j, lw, f, sidx, po))
                    sidx += 1
            for q0 in range(0, ncol, cfg.qcols):
                qn = min(cfg.qcols, ncol - q0)
                sub_ml = [
                    (cp - q0, lw, f, si, po)
                    for (cp, lw, f, si, po) in mlist
                    if q0 <= cp < q0 + qn
                ]
                chunk_entries[ki].append(len(call_meta))
                call_meta.append((sl0 + q0 * g, qn * g, w, q0, sl0, ncol))
                mm_meta.append(sub_ml)
    assert sidx == nsub_tot

    # ---- small constants ----
    sub = cfg.sub
    iota_rep = np.tile(
        np.arange(g, dtype=np.float32), (128, 1)
    ).astype(ml_dtypes.bfloat16)  # [128, g]
    W1bf = W1.astype(ml_dtypes.bfloat16)  # [nin, hid]
    W2p = np.zeros((cfg.hid, cfg.hid), dtype=np.float32)
    W2p[:, : cfg.outf] = W2
    W2bf = W2p.astype(ml_dtypes.bfloat16)
    b1col = b1.reshape(cfg.hid, 1).astype(np.float32)
    b2bc = np.zeros((128, cfg.hid), dtype=np.float32)
    b2bc[:, : cfg.outf] = b2[None, :]
    hasb1 = bool(np.any(b1))
    hasb2 = bool(np.any(b2))

    perp, nreg = cfg.perp, cfg.nreg
    in_maps = []
    for c in range(nc_):
        lo, hi = c * per, (c + 1) * per
        dslice = np.concatenate([dinv[lo:hi], np.ones(perp - per, np.float32)])
        mslice = np.concatenate([mult[lo:hi], np.zeros(perp - per, np.float32)])
        # feature-major own x slice, pre-scaled by dinv*mult (self term)
        xso = np.zeros((cfg.nin, perp), dtype=np.float32)
        xso[:, :per] = (x[lo:hi] * (dinv[lo:hi] * mult[lo:hi])[:, None]).T
        dbase = dslice if hasb1 else dslice * dslice
        dinv_bc = np.tile(dbase, (cfg.hid, 1)).astype(ml_dtypes.bfloat16)
        # node-major [128, nreg] scale tables for the layer-2 epilogue
        dcol = dslice.reshape(nreg, g).T.copy()
        scol = (dslice * mslice).reshape(nreg, g).T.copy()
        in_maps.append(
            {
                "xs": np.asarray(xs),
                "xsT_own": np.asarray(xso.astype(ml_dtypes.bfloat16)),
                "W1bf": np.asarray(W1bf),
                "W2bf": np.asarray(W2bf),
                "b1col": b1col,
                "b2bc": b2bc,
                "iota": np.asarray(iota_rep),
                "idx": idx_wrap[c],
                "lid2": np.asarray(lid2[c]),
                "dinv_bc": np.asarray(dinv_bc),
                "dinv_col": dcol,
                "self_col": scol,
            }
        )

    sched = dict(
        call_meta=call_meta,
        chunk_entries=chunk_entries,
        chunks=chunks,
        mm_meta=mm_meta,
        tot_slots=tot_slots,
        tot_cols=tot_cols,
        nsub_tot=nsub_tot,
        hasb1=hasb1,
        hasb2=hasb2,
    )
    return in_maps, sched


# ---------------------------------------------------------------- device side


def _build(cfg: Cfg, sched) -> bacc.Bacc:
    n, hid, g, gf, nwin, win = cfg.n, cfg.hid, cfg.g, cfg.gf, cfg.nwin, cfg.win
    ngf, per, perp, nreg, nlw = cfg.ngf, cfg.per, cfg.perp, cfg.nreg, cfg.nlw
    nin, of_ = cfg.nin, cfg.outf
    call_meta, chunks, mm_meta = sched["call_meta"], sched["chunks"], sched["mm_meta"]
    chunk_entries = sched["chunk_entries"]
    hasb1, hasb2 = sched["hasb1"], sched["hasb2"]
    tot_slots, nsub_tot = sched["tot_slots"], sched["nsub_tot"]
    max_call_cols = max((m[1] // g for m in call_meta), default=1)
    max_blk_cols = max((m[5] for m in call_meta), default=1)

    nc = bacc.Bacc("TRN2", target_bir_lowering=False, debug=False,
                   num_devices=cfg.ncores)

    xs_d = nc.dram_tensor("xs", [n, nin], BF16, kind="ExternalInput").ap()
    xso_d = nc.dram_tensor("xsT_own", [nin, perp], BF16, kind="ExternalInput").ap()
    W1bf = nc.dram_tensor("W1bf", [nin, hid], BF16, kind="ExternalInput").ap()
    W2bf = nc.dram_tensor("W2bf", [hid, hid], BF16, kind="ExternalInput").ap()
    b1col = nc.dram_tensor("b1col", [hid, 1], F32, kind="ExternalInput").ap()
    b2bc = nc.dram_tensor("b2bc", [128, hid], F32, kind="ExternalInput").ap()
    iota_d = nc.dram_tensor("iota", [128, g], BF16, kind="ExternalInput").ap()
    idx_d = nc.dram_tensor("idx", [128, tot_slots // 16], I16, kind="ExternalInput").ap()
    lid2_d = nc.dram_tensor("lid2", [128, 2 * nsub_tot], BF16, kind="ExternalInput").ap()
    dinvbc_d = nc.dram_tensor("dinv_bc", [hid, perp], BF16, kind="ExternalInput").ap()
    dinvcol_d = nc.dram_tensor("dinv_col", [128, nreg], F32, kind="ExternalInput").ap()
    selfcol_d = nc.dram_tensor("self_col", [128, nreg], F32, kind="ExternalInput").ap()

    out_d = nc.dram_tensor("out", [per, of_], F32, kind="ExternalOutput").ap()

    h2c_b = nc.dram_tensor("h2c_b", [per, of_], FP8).ap()   # AllGather input
    T3c = nc.dram_tensor("T3c", [n, of_], FP8, addr_space="Shared").ap()
    T3 = nc.dram_tensor("T3", [n, nin], BF16).ap()           # 256B-row table

    from concourse import library_config

    with tile.TileContext(nc) as tc, ExitStack() as ctx:
        nc.gpsimd.load_library(library_config.mlp)

        consts = ctx.enter_context(tc.tile_pool(name="consts", bufs=1))
        sb = ctx.enter_context(tc.tile_pool(name="sb", bufs=3))
        mtp = ctx.enter_context(tc.tile_pool(name="mtp", bufs=5))
        idxp = ctx.enter_context(tc.tile_pool(name="idxp", bufs=6))
        subp = ctx.enter_context(tc.tile_pool(name="subp", bufs=3))
        eptmp = ctx.enter_context(tc.tile_pool(name="eptmp", bufs=3))
        fep = ctx.enter_context(tc.tile_pool(name="fep", bufs=3))
        psum_acc = ctx.enter_context(tc.tile_pool(name="psuma", bufs=6, space="PSUM"))
        psum_mm = ctx.enter_context(tc.tile_pool(name="psummm", bufs=2, space="PSUM"))

        # resident constants
        w1_t = consts.tile([nin, hid], BF16)
        nc.sync.dma_start(w1_t[:], W1bf[:, :])
        w2_t = consts.tile([hid, hid], BF16)
        nc.sync.dma_start(w2_t[:], W2bf[:, :])
        b1_t = consts.tile([hid, 1], F32)
        nc.sync.dma_start(b1_t[:], b1col[:, :])
        b2_t = consts.tile([128, hid], F32)
        nc.sync.dma_start(b2_t[:], b2bc[:, :])
        iota_t = consts.tile([128, g], BF16)
        nc.sync.dma_start(iota_t[:], iota_d[:, :])
        dinvcol_t = consts.tile([128, nreg], F32)
        nc.sync.dma_start(dinvcol_t[:], dinvcol_d[:, :])
        selfcol_t = consts.tile([128, nreg], F32)
        nc.sync.dma_start(selfcol_t[:], selfcol_d[:, :])
        lid2_t = consts.tile([128, 2 * nsub_tot], BF16)
        nc.sync.dma_start(lid2_t[:], lid2_d[:, :])

        h2stage = consts.tile([128, nreg, hid], BF16)   # local h2 rows (nm)
        sh_g = consts.tile([128, nreg, of_], BF16)  # shifted logits staging
        sm_g = consts.tile([128, nreg, 1], F32)     # exp-sum staging


        # ---------------- batched layer-2 epilogue + log_softmax (per
        # region chunk, interleaved with layer-2 aggregation)
        def final_ep(a2c, r0, rn):
            a2 = a2c[:, :rn, :]
            nc.vector.tensor_tensor(
                out=a2,
                in0=a2,
                in1=dinvcol_t[:, r0 : r0 + rn]
                .unsqueeze(2)
                .broadcast_to([128, rn, hid]),
                op=ALU.mult,
            )
            o2 = fep.tile([128, rn, hid], F32, tag="fe2", name="o2")
            nc.vector.tensor_tensor(
                out=o2[:],
                in0=h2stage[:, r0 : r0 + rn, :],
                in1=selfcol_t[:, r0 : r0 + rn]
                .unsqueeze(2)
                .broadcast_to([128, rn, hid]),
                op=ALU.mult,
            )
            nc.vector.tensor_add(a2, a2, o2[:])
            if hasb2:
                nc.vector.tensor_tensor(
                    out=a2,
                    in0=a2,
                    in1=b2_t[:].unsqueeze(1).broadcast_to([128, rn, hid]),
                    op=ALU.add,
                )
            nmax = fep.tile([128, rn, 1], F32, tag="fm")
            nc.vector.tensor_reduce(
                nmax[:], a2[:, :, :of_], mybir.AxisListType.X,
                ALU.max, negate=True,
            )
            sh = sh_g[:, r0 : r0 + rn, :]
            nc.vector.tensor_tensor(
                out=sh,
                in0=a2[:, :, :of_],
                in1=nmax[:].broadcast_to([128, rn, of_]),
                op=ALU.add,
            )
            ex = o2[:, :rn, :of_]
            nc.scalar.activation(ex, sh, AF.Exp)
            nc.vector.tensor_reduce(
                sm_g[:, r0 : r0 + rn, :], ex, mybir.AxisListType.X, ALU.add
            )

        # deferred log_softmax finish: one Ln over all regions, then the
        # subtract + store in halves (overlaps the store with the subtract)
        def final_finish(lo, hi):
            nn = hi - lo
            ls = fep.tile([128, nreg, 1], F32, tag="fl")
            nc.scalar.activation(
                ls[:, lo:hi, :].rearrange("p q h -> p (q h)"),
                sm_g[:, lo:hi, :].rearrange("p q h -> p (q h)"),
                AF.Ln,
            )
            half = (nreg + 7) // 8
            for r0 in range(lo, hi, half):
                rn = min(half, hi - r0)
                fin = fep.tile([128, half, of_], F32, tag="fin", name="fin")
                nc.vector.tensor_tensor(
                    out=fin[:, :rn, :],
                    in0=sh_g[:, r0 : r0 + rn, :],
                    in1=ls[:, r0 : r0 + rn, :].broadcast_to([128, rn, of_]),
                    op=ALU.subtract,
                )
                nf = max(0, min(per // g - r0, rn))
                if nf:
                    nc.sync.dma_start(
                        out_d[r0 * g : (r0 + nf) * g, :].rearrange(
                            "(q p) f -> p q f", p=128
                        ),
                        fin[:, :nf, :],
                    )
                pi = per // g  # partial region index
                if r0 <= pi < r0 + rn and per % g:
                    nc.sync.dma_start(
                        out_d[pi * g : per, :], fin[: per % g, pi - r0, :]
                    )

        # ---------------- aggregation layers
        def agg_layer(layer: int, table_ap):
            # 512B pair-row view of the gather table ([n//2, 2*nin])
            tview = table_ap[:, :].rearrange("(q t) e -> q (t e)", t=2)
            npair = n // 2
            pending = [None]  # deferred epilogue of the previous chunk
            for ki, K in enumerate(chunks):
                # one PSUM bank per fat group (layer 1) / half bank (layer 2)
                if layer == 1:
                    banks = {
                        f: psum_acc.tile([128, gf], F32, tag="acc", name=f"a1_{f}")
                        for f in K
                    }

                    def acc_ap(f, lw):
                        return banks[f][:, lw * g : (lw + 1) * g]
                else:
                    bt = {}
                    for i in range(0, len(K), 2):
                        t = psum_acc.tile([128, 512], F32, tag="acc",
                                          name=f"a2_{K[i]}")
                        for j, f in enumerate(K[i : i + 2]):
                            bt[f] = (t, j)
                    banks = bt

                    def acc_ap(f, lw):
                        t, j = banks[f]
                        return t[:, (j * nlw + lw) * hid : (j * nlw + lw + 1) * hid]

                # per-bank first/last matmul bookkeeping
                def bank_key(f):
                    return id(banks[f]) if layer == 1 else id(banks[f][0])

                tot_bank = {}
                for ce in chunk_entries[ki]:
                    for cp, lw, f, si, po in mm_meta[ce]:
                        tot_bank[bank_key(f)] = tot_bank.get(bank_key(f), 0) + 1
                emitted = dict.fromkeys(tot_bank, 0)

                itb = None
                for ei, ce in enumerate(chunk_entries[ki]):
                    sl0, nsl, w2, q0, blk_sl0, blk_ncol = call_meta[ce]
                    mlist = mm_meta[ce]
                    w = ei  # flush key only
                    cols = nsl // g
                    wlen = min(cfg.pairw, npair - w2 * cfg.pairw)
                    if q0 == 0:
                        # one idx load per (chunk, window) block; sub-calls
                        # slice it (larger contiguous runs avoid the <512B
                        # DMA penalty)
                        itb = idxp.tile(
                            [128, max_blk_cols * 8], I16, tag="idx"
                        )
                        nc.sync.dma_start(
                            itb[:, : blk_ncol * 8],
                            idx_d[
                                :, blk_sl0 // 16 : blk_sl0 // 16 + blk_ncol * 8
                            ],
                        )
                    mt = mtp.tile([128, max_call_cols, 2 * nin], BF16, tag="m")
                    nc.gpsimd.dma_gather(
                        mt[:, :cols, :],
                        tview[w2 * cfg.pairw : w2 * cfg.pairw + wlen, :],
                        itb[:, q0 * 8 : q0 * 8 + cols * 8],
                        nsl,
                        nsl,
                        2 * nin,
                        single_packet=False,
                    )
                    # S slabs over this call's subcol range
                    si0 = mlist[0][3]
                    nsub = len(mlist)
                    for s0 in range(0, nsub, cfg.sub):
                        sc = min(cfg.sub, nsub - s0)
                        st_ = subp.tile([128, cfg.sub * g], BF16, tag="sel")
                        l2 = lid2_t[
                            :, 2 * (si0 + s0) : 2 * (si0 + s0 + sc)
                        ].rearrange("p (c t) -> p c t", t=2)
                        nc.vector.tensor_tensor(
                            out=st_[:, : sc * g].rearrange(
                                "p (c r t) -> p c r t", r=g // 2, t=2
                            ),
                            in0=iota_t[:, :]
                            .rearrange("p (r t) -> p r t", t=2)
                            .unsqueeze(1)
                            .broadcast_to([128, sc, g // 2, 2]),
                            in1=l2.unsqueeze(2).broadcast_to([128, sc, g // 2, 2]),
                            op=ALU.is_equal,
                        )
                        for k in range(sc):
                            cp, lw, f, si, po = mlist[s0 + k]
                            bk = bank_key(f)
                            first = emitted[bk] == 0
                            emitted[bk] += 1
                            last = emitted[bk] == tot_bank[bk]
                            if layer == 1:
                                nc.tensor.matmul(
                                    out=acc_ap(f, lw),
                                    lhsT=mt[:, cp, po * nin : (po + 1) * nin],
                                    rhs=st_[:, k * g : (k + 1) * g],
                                    start=first,
                                    stop=last,
                                )
                            else:
                                nc.tensor.matmul(
                                    out=acc_ap(f, lw),
                                    lhsT=st_[:, k * g : (k + 1) * g],
                                    rhs=mt[:, cp, po * nin : po * nin + hid],
                                    start=first,
                                    stop=last,
                                )
                    # flush the previous chunk's epilogue after this chunk's
                    # first window so it overlaps the following gathers
                    if w == 0 and pending[0] is not None:
                        pending[0]()
                        pending[0] = None
                        if layer == 2 and K is chunks[-1]:
                            # all earlier chunks' exp-sums are now emitted:
                            # finish their log_softmax while this chunk runs
                            final_finish(0, K[0] * nlw)

                # epilogues for chunk K (deferred by one window)
                def do_ep(K=K, banks=banks):
                    if layer == 2:
                        a2c = fep.tile(
                            [128, cfg.chunk_f * nlw, hid], F32, tag="a2c",
                            name="a2c",
                        )
                    for f in K:
                        c0 = f * gf
                        if layer == 1:
                        dv = eptmp.tile([hid, gf], BF16, tag="dv")
                        nc.scalar.dma_start(dv[:], dinvbc_d[:, c0 : c0 + gf])
                        accS = eptmp.tile([128, gf], BF16, tag="ep0")
                        nc.vector.tensor_copy(accS[:], banks[f][:])
                        hp = psum_mm.tile([hid, gf], F32, tag="mm", name="h1pre")
                        nc.tensor.matmul(
                            out=hp[:], lhsT=w1_t[:], rhs=accS[:],
                            start=True, stop=False,
                        )
                        nc.tensor.matmul(
                            out=hp[:], lhsT=ident_t[:],
                            rhs=t1own[:, c0 : c0 + gf],
                            start=False, stop=True,
                        )
                        if hasb1:
                            t2_ = eptmp.tile([hid, gf], F32, tag="ep2")
                            nc.vector.tensor_mul(t2_[:], hp[:], dv[:])
                            t3_ = eptmp.tile([hid, gf], F32, tag="ep3")
                            nc.scalar.activation(
                                t3_[:], t2_[:], AF.Relu, bias=b1_t[:, :1]
                            )
                            t2p = eptmp.tile([hid, gf], BF16, tag="ep4")
                            nc.vector.tensor_mul(t2p[:], t3_[:], dv[:])
                        else:
                            # b1 == 0: relu(dinv*u)*dinv == relu(u)*dinv^2
                            # (dv holds dinv^2 in this mode)
                            t3_ = eptmp.tile([hid, gf], BF16, tag="ep3")
                            nc.scalar.activation(t3_[:], hp[:], AF.Relu)
                            t2p = eptmp.tile([hid, gf], BF16, tag="ep4")
                            nc.vector.tensor_mul(t2p[:], t3_[:], dv[:])
                        # W2 matmuls -> node-major h2 staging
                        pw = psum_mm.tile([128, nlw * hid], F32, tag="mm", name="pw")
                        for q in range(nlw):
                            nc.tensor.matmul(
                                out=pw[:, q * hid : (q + 1) * hid],
                                lhsT=t2p[:, q * g : (q + 1) * g],
                                rhs=w2_t[:],
                                start=True,
                                stop=True,
                            )
                        nc.vector.tensor_copy(
                            h2stage[:, f * nlw : (f + 1) * nlw, :].rearrange(
                                "p q h -> p (q h)"
                            ),
                            pw[:],
                        )
                        h2f8 = fep.tile([128, nlw, hid], FP8, tag="h8")
                        nc.vector.tensor_copy(
                            h2f8[:].rearrange("p q h -> p (q h)"), pw[:]
                        )
                        rows = min(gf, per - f * gf)
                        nfq = rows // g
                        if nfq:
                            nc.scalar.dma_start(
                                h2c_b[f * gf : f * gf + nfq * g, :].rearrange(
                                    "(q p) h -> p q h", p=128
                                ),
                                h2f8[:, :nfq, :of_],
                            )
                        if rows % g:
                            nc.scalar.dma_start(
                                h2c_b[f * gf + nfq * g : f * gf + rows, :],
                                h2f8[: rows % g, nfq, :of_],
                            )
                    else:
                        t, j = banks[f]
                        jj = f - K[0]
                        nc.vector.tensor_copy(
                            a2c[:, jj * nlw : (jj + 1) * nlw, :].rearrange(
                                "p q h -> p (q h)"
                            ),
                            t[:, j * nlw * hid : (j + 1) * nlw * hid],
                        )
                if layer == 2:
                    final_ep(a2c, K[0] * nlw, len(K) * nlw)

        _phases = int(os.environ.get("GCN_PHASES", "4"))
        if _phases >= 2:
            agg_layer(1, xs_d)

        if _phases >= 3 and not os.environ.get("GCN_NO_COLL"):
            nc.gpsimd.collective_compute(
                "AllGather",
                ALU.bypass,
                replica_groups=[list(range(cfg.ncores))],
                ins=[h2c_b.opt()],
                outs=[T3c.opt()],
            )
            # expand compact fp8 rows into the bf16 256B-row gather table:
            # DMA in (fused q-view), Act copy converts fp8->bf16, DMA out.
            cq = 13
            r0 = 0
            while r0 < n:
                rr = min(cq * g, n - r0)
                q = rr // g
                if q:
                    f8 = sb.tile([128, cq, of_], FP8, tag="x8")
                    nc.sync.dma_start(
                        f8[:, :q, :],
                        T3c[r0 : r0 + q * g, :].rearrange(
                            "(q p) h -> p q h", p=128
                        ),
                    )
                    bo = sb.tile([128, cq, of_], BF16, tag="xb")
                    nc.vector.tensor_copy(bo[:, :q, :], f8[:, :q, :])
                    nc.sync.dma_start(
                        T3[r0 : r0 + q * g, :of_].rearrange(
                            "(q p) h -> p q h", p=128
                        ),
                        bo[:, :q, :],
                    )
                    r0 += q * g
                else:
                    f8 = sb.tile([128, cq, of_], FP8, tag="x8")
                    nc.sync.dma_start(f8[: n - r0, 0, :], T3c[r0:n, :])
                    bo = sb.tile([128, cq, of_], BF16, tag="xb")
                    nc.vector.tensor_copy(bo[: n - r0, 0, :], f8[: n - r0, 0, :])
                    nc.sync.dma_start(T3[r0:n, :of_], bo[: n - r0, 0, :])
                    r0 = n

        if _phases >= 4:
            agg_layer(2, T3)
            final_finish(chunks[-1][0] * nlw, nreg)

    nc.compile()
    return nc


# ---------------------------------------------------------------- entry


def kernel(x, edge_index, W1, b1, W2, b2, cfg: Cfg | None = None, _run=None):
    cfg = cfg or Cfg()
    in_maps, sched = _preprocess(
        np.asarray(x), np.asarray(edge_index), np.asarray(W1), np.asarray(b1),
        np.asarray(W2), np.asarray(b2), cfg
    )
    nc = _build(cfg, sched)
    if _run is not None:  # test hook (e.g. simulator)
        results = _run(nc, in_maps)
    else:
        results = run_bass_kernel_spmd(
            nc, in_maps, core_ids=list(range(cfg.ncores))
        ).results
    out = np.concatenate([results[c]["out"] for c in range(cfg.ncores)], axis=0)
    return out.astype(np.float32)
